# revision 1
# baseline (speedup 1.0000x reference)
"""CARAFE ghost-conv kernel for 8 Trainium2 NeuronCores.

Self-contained: takes FULL inputs (as in setup_inputs()), returns FULL output
(4, 256, 160, 160) float32.

Sharding: 8 cores = 4 batches x 2 H-halves (data-parallel, halo'd on host).
Per core: input rows [40*half-6, 40*half+46) (52 rows, zero-padded outside the
image), W padded 80->84 (cols 2..81 valid). Output rows [80*half, 80*half+80).

Pipeline per core (all resident in SBUF):
  S1 down_cv1 1x1 256->32 (PE f32r) + BN+SiLU (ACT)
  S2 down_cv2 dw5x5 (PE diag-matmul bf16) + BN+SiLU
  S3 enc_cv1 3x3 64->18 (PE, 9 shifted-tap f32r matmuls) + BN+SiLU
  S4 enc_cv2 dw5x5 (PE diag bf16) + BN+SiLU
  S5 softmax over k=9 -> kt, pixel-major via PE transposes
  S6 Z = out_cv1-conv(x) at low res (PE f32r; CARAFE is fused through the 1x1)
     + 3 horizontally-shifted transposed bf16 copies ZT_d (pixel-major)
  S7 CARAFE: o_pre[w,rh,r,:] = sum_k kt[w,rh,k*4+r] * ZT_{1+dw}[w, rh+dh, :]
     (scalar_tensor_tensor FMAs, split across DVE and GPSIMD)
  S8 transpose back to channel-major, interleave R=2 phases -> o2c (128,84,164)
     + BN+SiLU of out_cv1
  S9 out_cv2 dw5x5 on the 160-grid (PE diag bf16 for most rows, DVE/GPSIMD STT
     for the rest) + BN+SiLU -> output channels 128..255
  S10 DMA o2c valid window -> output channels 0..127
"""

import numpy as np
import ml_dtypes

import concourse.bacc as bacc
import concourse.bass as bass
import concourse.tile as tile
from concourse import mybir
from concourse.bass_utils import run_bass_kernel_spmd
from concourse.masks import make_identity

F32 = mybir.dt.float32
F32R = mybir.dt.float32r
BF16 = mybir.dt.bfloat16
AF = mybir.ActivationFunctionType
ALU = mybir.AluOpType
AX = mybir.AxisListType

STOP_AFTER = None   # None | 'S4' | 'S6' | 'S8' (profiling aid)
EPS = 1e-5
WP = 84          # padded low-res width
ROWS = 52        # local input rows (valid image rows at local 6..45)
NKT = 42         # kt / o rows (local rows 5..46)
NZ = 44          # Z rows kept (local rows 4..47)
WO = 164         # padded hi-res width
HO = 84          # hi-res rows (output rows 80*half-2 .. 80*half+82)

S9_PE_ROWS = 80


def _act(nc, out, in_, func, scale=1.0, bias=0.0):
    nc.scalar.activation(out=out, in_=in_, func=func, scale=scale, bias=bias)


def build_kernel():
    nc = bacc.Bacc("TRN2", target_bir_lowering=False, debug=False, num_devices=8)

    d = {}
    d["x_d"] = nc.declare_dram_parameter("x", [256, ROWS, WP], BF16, isOutput=False)
    d["edge_d"] = nc.declare_dram_parameter("edge", [128, 2], F32, isOutput=False)
    d["wdn1_d"] = nc.declare_dram_parameter("wdn1", [256, 32], BF16, isOutput=False)
    d["bdn1_d"] = nc.declare_dram_parameter("bdn1", [32, 2], F32, isOutput=False)
    d["ddn2_d"] = nc.declare_dram_parameter("ddn2", [25, 128, 128], BF16, isOutput=False)
    d["bdn2_d"] = nc.declare_dram_parameter("bdn2", [128, 2], F32, isOutput=False)
    d["wenc_d"] = nc.declare_dram_parameter("wenc", [9, 64, 18], BF16, isOutput=False)
    d["benc1_d"] = nc.declare_dram_parameter("benc1", [18, 2], F32, isOutput=False)
    d["denc2_d"] = nc.declare_dram_parameter("denc2", [25, 126, 126], BF16, isOutput=False)
    d["benc2_d"] = nc.declare_dram_parameter("benc2", [126, 2], F32, isOutput=False)
    d["wout1_d"] = nc.declare_dram_parameter("wout1", [256, 128], BF16, isOutput=False)
    d["bout1_d"] = nc.declare_dram_parameter("bout1", [128, 2], F32, isOutput=False)
    d["dout2_d"] = nc.declare_dram_parameter("dout2", [25, 128, 128], BF16, isOutput=False)
    d["wto2_d"] = nc.declare_dram_parameter("wto2", [128, 25], F32, isOutput=False)
    d["bout2_d"] = nc.declare_dram_parameter("bout2", [128, 2], F32, isOutput=False)
    d["out_d"] = nc.declare_dram_parameter("out", [256, 80, 160], F32, isOutput=True)

    with tile.TileContext(nc) as tc:
        _emit(nc, tc, d)
    nc.compile()
    return nc


def _emit(nc, tc, d):
    x_d, out_d = d["x_d"], d["out_d"]

    from contextlib import ExitStack
    ctx = ExitStack()
    with ctx:
        consts = ctx.enter_context(tc.tile_pool(name="consts", bufs=1))
        mid = ctx.enter_context(tc.tile_pool(name="mid", bufs=1))
        stage = ctx.enter_context(tc.tile_pool(name="stage", bufs=4))
        slab = ctx.enter_context(tc.tile_pool(name="slab", bufs=6))
        psA = ctx.enter_context(tc.tile_pool(name="psA", bufs=4, space="PSUM"))
        psT = ctx.enter_context(tc.tile_pool(name="psT", bufs=3, space="PSUM"))

        # ---- constants ---------------------------------------------------
        ident = consts.tile([128, 128], F32)
        make_identity(nc, ident[:])
        identb = consts.tile([128, 128], BF16)
        nc.gpsimd.tensor_copy(identb[:], ident[:])

        edge = consts.tile([128, 2], F32)
        nc.sync.dma_start(out=edge[:], in_=d["edge_d"][:])
        et, eb = edge[:, 0:1], edge[:, 1:2]

        wdn1 = consts.tile([128, 2, 32], BF16)
        nc.sync.dma_start(out=wdn1[:, 0, :], in_=d["wdn1_d"][0:128, :])
        nc.sync.dma_start(out=wdn1[:, 1, :], in_=d["wdn1_d"][128:256, :])
        bdn1 = consts.tile([32, 2], F32)
        nc.sync.dma_start(out=bdn1[:], in_=d["bdn1_d"][:])
        ddn2p = consts.tile([128, 25, 128], BF16)
        nc.sync.dma_start(out=ddn2p[:], in_=d["ddn2_d"][:].rearrange("t k m -> k t m"))
        bdn2 = consts.tile([128, 2], F32)
        nc.sync.dma_start(out=bdn2[:], in_=d["bdn2_d"][:])
        wenc = consts.tile([64, 9, 18], BF16)
        nc.sync.dma_start(out=wenc[:], in_=d["wenc_d"][:].rearrange("t k m -> k t m"))
        benc1 = consts.tile([18, 2], F32)
        nc.sync.dma_start(out=benc1[:], in_=d["benc1_d"][:])
        denc2p = consts.tile([126, 25, 126], BF16)
        nc.sync.dma_start(out=denc2p[:], in_=d["denc2_d"][:].rearrange("t k m -> k t m"))
        benc2 = consts.tile([126, 2], F32)
        nc.sync.dma_start(out=benc2[:], in_=d["benc2_d"][:])
        wout1 = consts.tile([128, 2, 128], BF16)
        nc.sync.dma_start(out=wout1[:, 0, :], in_=d["wout1_d"][0:128, :])
        nc.sync.dma_start(out=wout1[:, 1, :], in_=d["wout1_d"][128:256, :])
        bout1 = consts.tile([128, 2], F32)
        nc.sync.dma_start(out=bout1[:], in_=d["bout1_d"][:])
        dout2 = consts.tile([128, 25, 128], BF16)
        nc.sync.dma_start(out=dout2[:], in_=d["dout2_d"][:].rearrange("t k m -> k t m"))
        wto2 = consts.tile([128, 25], F32)
        nc.sync.dma_start(out=wto2[:], in_=d["wto2_d"][:])
        bout2 = consts.tile([128, 2], F32)
        nc.sync.dma_start(out=bout2[:], in_=d["bout2_d"][:])

        # mid-lived tensors
        kt = mid.tile([84, NKT, 36], F32)
        zt1 = mid.tile([84, NZ, 128], BF16)
        zt2 = mid.tile([84, NZ, 128], BF16)
        zt3 = mid.tile([84, NZ, 128], BF16)
        zts = {1: zt1, 2: zt2, 3: zt3}

        with tc.tile_pool(name="early", bufs=1) as early:
            x0 = early.tile([128, ROWS, WP], BF16)
            x1 = early.tile([128, ROWS, WP], BF16)
            nc.sync.dma_start(out=x0[:], in_=x_d[0:128])
            nc.sync.dma_start(out=x1[:], in_=x_d[128:256])
            down_t = early.tile([64, ROWS * WP + 8], BF16)
            down = down_t[:, 4:4 + ROWS * WP].rearrange("p (r w) -> p r w", w=WP)
            e = early.tile([36, ROWS, WP], F32)
            e1bf = early.tile([18, ROWS, WP], BF16)
            zc = early.tile([128, NZ, WP], BF16)
            nc.gpsimd.memset(down_t[:, 0:4], 0.0)
            nc.gpsimd.memset(down_t[:, 4 + ROWS * WP:], 0.0)

            def win(flat, p0, p1, off, rr, w):
                return flat[p0:p1, off:off + rr * w].rearrange(
                    "p (r w) -> p r w", w=w)

            y1 = down[0:32]
            nc.vector.memset(down[32:64, 0:2, :], 0.0)
            nc.vector.memset(down[32:64, 50:52, :], 0.0)


            # ---- S1: down_cv1 + BN + SiLU -------------------------------
            for c0 in range(0, ROWS, 6):
                rr = min(6, ROWS - c0)
                ps = psA.tile([128, 6, WP], F32, tag="ps")
                nc.tensor.matmul(ps[0:32, 0:rr, :], wdn1[:, 0, :],
                                 x0[:, c0:c0 + rr, :], start=True, stop=False)
                nc.tensor.matmul(ps[0:32, 0:rr, :], wdn1[:, 1, :],
                                 x1[:, c0:c0 + rr, :], start=False, stop=True)
                _act(nc, y1[:, c0:c0 + rr, :], ps[0:32, 0:rr, :], AF.Silu,
                     scale=bdn1[:, 0:1], bias=bdn1[:, 1:2])
            nc.vector.tensor_scalar_mul(y1[:, 0:6, :], y1[:, 0:6, :], et[0:32])
            nc.vector.tensor_scalar_mul(y1[:, 46:52, :], y1[:, 46:52, :], eb[0:32])
            nc.vector.memset(y1[:, :, 0:2], 0.0)
            nc.vector.memset(y1[:, :, 82:84], 0.0)

            # ---- S2: down_cv2 (diag bf16, 4 row-groups packed) ----------
            # group g (partitions 32g..32g+32) holds y1 rows [12g, 12g+16);
            # its outputs are rows [12g+2, 12g+14)
            y1s_t = early.tile([128, 16 * WP + 8], BF16)
            nc.gpsimd.memset(y1s_t[:, 0:4], 0.0)
            nc.gpsimd.memset(y1s_t[:, 4 + 16 * WP:], 0.0)
            for g in range(4):
                nc.sync.dma_start(
                    out=y1s_t[32 * g:32 * g + 32, 4:4 + 16 * WP],
                    in_=down_t[0:32, 4 + 12 * g * WP:4 + (12 * g + 16) * WP])
            for j in range(2):
                ps = psA.tile([128, 6, WP], F32, tag="ps")
                for t in range(25):
                    dh, dw = divmod(t, 5)
                    off = 4 + (6 * j + dh) * WP + (dw - 2)
                    nc.tensor.matmul(
                        ps[:, 0:6, :], ddn2p[:, t, :],
                        win(y1s_t, 0, 128, off, 6, WP),
                        start=(t == 0), stop=(t == 24))
                st = stage.tile([128, 6, WP], BF16, tag="sty2")
                _act(nc, st[:], ps[:], AF.Silu,
                     scale=bdn2[:, 0:1], bias=bdn2[:, 1:2])
                # edge masks: rows 2..5 (group0 chunk0 rows 0..3, et),
                # rows 46..49 (group3 chunk1 rows 2..5, eb)
                if j == 0:
                    nc.vector.tensor_scalar_mul(st[0:32, 0:4, :], st[0:32, 0:4, :],
                                                et[0:32])
                else:
                    nc.vector.tensor_scalar_mul(st[96:128, 2:6, :], st[96:128, 2:6, :],
                                                eb[96:128])
                nc.vector.memset(st[:, :, 0:2], 0.0)
                nc.vector.memset(st[:, :, 82:84], 0.0)
                for g in range(4):
                    nc.sync.dma_start(
                        out=down[32:64, 12 * g + 2 + 6 * j:12 * g + 8 + 6 * j, :],
                        in_=st[32 * g:32 * g + 32, :, :])

            # ---- S3: enc_cv1 (9 taps) + BN + SiLU -----------------------
            for c0 in range(3, 49, 6):
                rr = min(6, 49 - c0)
                ps = psA.tile([128, 6, WP], F32, tag="ps")
                for t in range(9):
                    dh, dw = divmod(t, 3)
                    off = 4 + (c0 - 1 + dh) * WP + (dw - 1)
                    nc.tensor.matmul(
                        ps[0:18, 0:rr, :], wenc[:, t, :],
                        win(down_t, 0, 64, off, rr, WP),
                        start=(t == 0), stop=(t == 8))
                _act(nc, e[0:18, c0:c0 + rr, :], ps[0:18, 0:rr, :], AF.Silu,
                     scale=benc1[:, 0:1], bias=benc1[:, 1:2])
            e1 = e[0:18]
            nc.vector.tensor_scalar_mul(e1[:, 3:6, :], e1[:, 3:6, :], et[0:18])
            nc.vector.tensor_scalar_mul(e1[:, 46:49, :], e1[:, 46:49, :], eb[0:18])
            nc.vector.memset(e1[:, :, 0:2], 0.0)
            nc.vector.memset(e1[:, :, 82:84], 0.0)
            nc.vector.memset(e1[:, 2:3, :], 0.0)
            nc.vector.memset(e1[:, 49:50, :], 0.0)
            nc.gpsimd.tensor_copy(e1bf[:, 2:50, :], e1[:, 2:50, :])

            # ---- S4: enc_cv2 (diag bf16, 7 row-groups packed) -----------
            # group g (partitions 18g..18g+18) holds e1 rows [6g+3, 6g+13);
            # outputs rows [6g+5, 6g+11)
            e1s_t = early.tile([128, 10 * WP + 8], BF16)
            nc.gpsimd.memset(e1s_t[:, 0:4], 0.0)
            nc.gpsimd.memset(e1s_t[:, 4 + 10 * WP:], 0.0)
            for g in range(7):
                nc.sync.dma_start(
                    out=e1s_t[18 * g:18 * g + 18, 4:4 + 10 * WP],
                    in_=e1bf[:, 6 * g + 3:6 * g + 13, :])
            ps = psA.tile([128, 6, WP], F32, tag="ps")
            for t in range(25):
                dh, dw = divmod(t, 5)
                off = 4 + dh * WP + (dw - 2)
                nc.tensor.matmul(
                    ps[0:126, 0:6, :], denc2p[:, t, :],
                    win(e1s_t, 0, 126, off, 6, WP),
                    start=(t == 0), stop=(t == 24))
            st = stage.tile([126, 6, WP], F32, tag="ste2")
            _act(nc, st[:], ps[0:126, :, :], AF.Silu,
                 scale=benc2[:, 0:1], bias=benc2[:, 1:2])
            for g in range(7):
                nc.sync.dma_start(out=e[18:36, 6 * g + 5:6 * g + 11, :],
                                  in_=st[18 * g:18 * g + 18, :, :])

            if STOP_AFTER == 'S4':
                return
            # ---- S5: softmax -> kt (pixel-major, per row) ---------------
            for rh in range(NKT):
                row = 5 + rh
                pt = psT.tile([80, 36], F32, tag="pt")
                nc.tensor.transpose(pt[:], e[0:36, row, 2:82], ident[0:36, 0:36])
                erow = stage.tile([80, 36], F32, tag="erow")
                _act(nc, erow[:], pt[0:80, 0:36], AF.Exp)
                srow = stage.tile([80, 4], F32, tag="srow")
                nc.vector.tensor_reduce(
                    srow[:], erow[:].rearrange("w (k r) -> w r k", k=9),
                    axis=AX.X, op=ALU.add)
                nc.vector.reciprocal(srow[:], srow[:])
                nc.vector.tensor_tensor(
                    kt[0:80, rh, :].rearrange("w (k r) -> w k r", k=9),
                    erow[:].rearrange("w (k r) -> w k r", k=9),
                    srow[:].unsqueeze(1).to_broadcast((80, 9, 4)),
                    op=ALU.mult)

            # ---- S6: Z + shifted transposed copies ----------------------
            for c0 in range(0, NZ, 6):
                rr = min(6, NZ - c0)
                ps = psA.tile([128, 6, WP], F32, tag="ps")
                nc.tensor.matmul(ps[:, 0:rr, :], wout1[:, 0, :],
                                 x0[:, 4 + c0:4 + c0 + rr, :], start=True, stop=False)
                nc.tensor.matmul(ps[:, 0:rr, :], wout1[:, 1, :],
                                 x1[:, 4 + c0:4 + c0 + rr, :], start=False, stop=True)
                _act(nc, zc[:, c0:c0 + rr, :], ps[:, 0:rr, :], AF.Copy)
            for zr in range(NZ):
                for di, dd in enumerate((1, 2, 3)):
                    pt = psT.tile([80, 128], BF16, tag="pt")
                    nc.tensor.transpose(pt[:], zc[:, zr, dd:dd + 80], identb[:])
                    if (zr * 3 + di) % 2 == 0:
                        _act(nc, zts[dd][0:80, zr, :], pt[0:80, 0:128], AF.Copy)
                    else:
                        nc.vector.tensor_copy(zts[dd][0:80, zr, :], pt[0:80, 0:128])

        if STOP_AFTER == 'S6':
            return
        # early pool freed here
        with tc.tile_pool(name="late", bufs=1) as late:
            o2c = late.tile([128, HO, WO], F32)
            o2bf_t = late.tile([128, HO * WO + 8], BF16)
            o2bf = o2bf_t[:, 4:4 + HO * WO].rearrange("p (r w) -> p r w", w=WO)
            nc.gpsimd.memset(o2bf_t[:, 0:4], 0.0)
            nc.gpsimd.memset(o2bf_t[:, 4 + HO * WO:], 0.0)

            # ---- S7 + S8: CARAFE FMAs, transpose back, assemble ---------
            nc.vector.memset(o2c[:, :, 0:2], 0.0)
            nc.vector.memset(o2c[:, :, 162:164], 0.0)
            for rh in range(NKT):
                # r1=0 (r=0,1): DVE scalar-FMA chains + PE transposes
                ptd = psT.tile([128, 2, 80], BF16, tag="pt")
                for r2 in range(2):
                    r = r2
                    acc = slab.tile([80, 128], BF16, tag="acc")
                    for k in range(9):
                        dh, dwp = divmod(k, 3)
                        src = zts[1 + dwp][0:80, rh + dh, :]
                        sc = kt[0:80, rh, 4 * k + r:4 * k + r + 1]
                        if k == 0:
                            nc.vector.tensor_scalar_mul(acc[:], src, sc)
                        else:
                            nc.vector.scalar_tensor_tensor(
                                out=acc[:], in0=src, scalar=sc, in1=acc[:],
                                op0=ALU.mult, op1=ALU.add)
                    nc.tensor.transpose(ptd[:, r2, :], acc[:], identb[0:80, 0:80])
                _act(nc,
                     o2c[:, 2 * rh, 2:162].rearrange("p (w q) -> p q w", q=2),
                     ptd[0:128, :, :], AF.Silu,
                     scale=bout1[:, 0:1], bias=bout1[:, 1:2])
                # r1=1 (r=2,3): GPSIMD diag build + PE scaled-transpose-accum
                dg = slab.tile([80, 2, 9, 80], BF16, tag="dg")
                nc.gpsimd.tensor_tensor(
                    dg[:],
                    identb[0:80, 0:80].unsqueeze(1).unsqueeze(1)
                        .to_broadcast((80, 2, 9, 80)),
                    kt[0:80, rh:rh + 1, :]
                        .rearrange("p one (k f) -> p one k f", k=9)[:, 0, :, 2:4]
                        .rearrange("p k q -> p q k")
                        .unsqueeze(3).to_broadcast((80, 2, 9, 80)),
                    op=ALU.mult)
                ptp = psT.tile([128, 2, 80], F32, tag="pt")
                for r2 in range(2):
                    for k in range(9):
                        dh, dwp = divmod(k, 3)
                        nc.tensor.matmul(
                            ptp[:, r2, :], zts[1 + dwp][0:80, rh + dh, :],
                            dg[:, r2, k, :], start=(k == 0), stop=(k == 8))
                _act(nc,
                     o2c[:, 2 * rh + 1, 2:162].rearrange("p (w q) -> p q w", q=2),
                     ptp[0:128, :, :], AF.Silu,
                     scale=bout1[:, 0:1], bias=bout1[:, 1:2])
                # incremental finalize + bf16 cast of this row-pair so S9 can
                # start before the whole CARAFE output exists
                if rh == 0:
                    nc.vector.tensor_scalar_mul(o2c[:, 0:2, :], o2c[:, 0:2, :], et)
                if rh == NKT - 1:
                    nc.vector.tensor_scalar_mul(o2c[:, 82:84, :], o2c[:, 82:84, :], eb)
                nc.gpsimd.tensor_copy(o2bf[:, 2 * rh:2 * rh + 2, :],
                                      o2c[:, 2 * rh:2 * rh + 2, :])

            if STOP_AFTER == 'S8':
                return
            # ---- S10: output channels 0..127 ----------------------------
            nc.sync.dma_start(out=out_d[0:128], in_=o2c[:, 2:82, 2:162])

            # ---- S9: out_cv2 + BN + SiLU -> channels 128..255 -----------
            for g0 in range(2, 82, 9):
                gr = min(9, 82 - g0)
                st = stage.tile([128, 9, WO], F32, tag="st9")
                for c0 in range(g0, g0 + gr, 3):
                    rr = min(3, g0 + gr - c0)
                    ps = psA.tile([128, 3, WO], F32, tag="ps")
                    for t in range(25):
                        dh, dw = divmod(t, 5)
                        off = 4 + (c0 - 2 + dh) * WO + (dw - 2)
                        nc.tensor.matmul(
                            ps[:, 0:rr, :], dout2[:, t, :],
                            win(o2bf_t, 0, 128, off, rr, WO),
                            start=(t == 0), stop=(t == 24))
                    _act(nc, st[:, c0 - g0:c0 - g0 + rr, :], ps[:, 0:rr, :], AF.Silu,
                         scale=bout2[:, 0:1], bias=bout2[:, 1:2])
                nc.sync.dma_start(out=out_d[128:256, g0 - 2:g0 - 2 + gr, :],
                                  in_=st[:, 0:gr, 2:162])


# ---------------------------------------------------------------------------
# host side
# ---------------------------------------------------------------------------

_NC_CACHE = {}


def _get_nc():
    if "nc" not in _NC_CACHE:
        _NC_CACHE["nc"] = build_kernel()
    return _NC_CACHE["nc"]


def _bn2(g, b, m, v):
    inv = (g / np.sqrt(v + EPS)).astype(np.float32)
    beta = (b - m * inv).astype(np.float32)
    return np.stack([inv, beta], axis=1).astype(np.float32)


def _diag_taps(w, c, rep=1):
    taps = np.tile(w.reshape(c, 25).T, (1, rep))      # (25, c*rep)
    n = c * rep
    out = np.zeros((25, n, n), np.float32)
    idx = np.arange(n)
    out[:, idx, idx] = taps
    return out.astype(ml_dtypes.bfloat16)


def _tile_bn(bn, rep):
    return np.tile(bn, (rep, 1))


def prep_in_maps(inputs):
    inp = {k: np.asarray(v) for k, v in inputs.items()}
    x = inp["x"].astype(np.float32)

    common = dict(
        wdn1=np.ascontiguousarray(inp["down_cv1_w"].reshape(32, 256).T).astype(ml_dtypes.bfloat16),
        bdn1=_bn2(inp["down_cv1_g"], inp["down_cv1_b"], inp["down_cv1_m"], inp["down_cv1_v"]),
        ddn2=_diag_taps(inp["down_cv2_w"], 32, rep=4),
        bdn2=_tile_bn(_bn2(inp["down_cv2_g"], inp["down_cv2_b"], inp["down_cv2_m"], inp["down_cv2_v"]), 4),
        wenc=np.ascontiguousarray(inp["enc_cv1_w"].reshape(18, 64, 9).transpose(2, 1, 0)).astype(ml_dtypes.bfloat16),
        benc1=_bn2(inp["enc_cv1_g"], inp["enc_cv1_b"], inp["enc_cv1_m"], inp["enc_cv1_v"]),
        denc2=_diag_taps(inp["enc_cv2_w"], 18, rep=7),
        benc2=_tile_bn(_bn2(inp["enc_cv2_g"], inp["enc_cv2_b"], inp["enc_cv2_m"], inp["enc_cv2_v"]), 7),
        wout1=np.ascontiguousarray(inp["out_cv1_w"].reshape(128, 256).T).astype(ml_dtypes.bfloat16),
        bout1=_bn2(inp["out_cv1_g"], inp["out_cv1_b"], inp["out_cv1_m"], inp["out_cv1_v"]),
        dout2=_diag_taps(inp["out_cv2_w"], 128),
        wto2=np.ascontiguousarray(inp["out_cv2_w"].reshape(128, 25)).astype(np.float32),
        bout2=_bn2(inp["out_cv2_g"], inp["out_cv2_b"], inp["out_cv2_m"], inp["out_cv2_v"]),
    )

    in_maps = []
    for s in range(8):
        n, half = s // 2, s % 2
        h0 = 40 * half
        xs = np.zeros((256, ROWS, WP), ml_dtypes.bfloat16)
        src_lo = max(0, h0 - 6)
        src_hi = min(80, h0 + 46)
        xs[:, src_lo - (h0 - 6):src_hi - (h0 - 6), 2:82] = x[n, :, src_lo:src_hi, :]
        edge = np.zeros((128, 2), np.float32)
        edge[:, 0] = 0.0 if half == 0 else 1.0
        edge[:, 1] = 1.0 if half == 0 else 0.0
        in_maps.append(dict(x=xs, edge=edge, **common))
    return in_maps


def kernel(**inputs):
    in_maps = prep_in_maps(inputs)
    nc = _get_nc()
    res = run_bass_kernel_spmd(nc, in_maps, list(range(8)))
    _NC_CACHE["last_result"] = res

    out = np.empty((4, 256, 160, 160), np.float32)
    for s in range(8):
        n, half = s // 2, s % 2
        out[n, :, 80 * half:80 * half + 80, :] = res.results[s]["out"]
    return out



# revision 5
# speedup vs baseline: 1.3143x; 1.3143x over previous
"""CARAFE ghost-conv kernel for 8 Trainium2 NeuronCores.

Self-contained: takes FULL inputs (as in setup_inputs()), returns FULL output
(4, 256, 160, 160) float32.

Sharding: 8 cores = 4 batches x 2 H-halves (data-parallel, halo'd on host).
Per core: input rows [40*half-6, 40*half+46) (52 rows, zero-padded outside the
image), W padded 80->84 (cols 2..81 valid). Output rows [80*half, 80*half+80).

Pipeline per core (all resident in SBUF):
  S1 down_cv1 1x1 256->32 (PE f32r) + BN+SiLU (ACT)
  S2 down_cv2 dw5x5 (PE diag-matmul bf16) + BN+SiLU
  S3 enc_cv1 3x3 64->18 (PE, 9 shifted-tap f32r matmuls) + BN+SiLU
  S4 enc_cv2 dw5x5 (PE diag bf16) + BN+SiLU
  S5 softmax over k=9 -> kt, pixel-major via PE transposes
  S6 Z = out_cv1-conv(x) at low res (PE f32r; CARAFE is fused through the 1x1)
     + 3 horizontally-shifted transposed bf16 copies ZT_d (pixel-major)
  S7 CARAFE: o_pre[w,rh,r,:] = sum_k kt[w,rh,k*4+r] * ZT_{1+dw}[w, rh+dh, :]
     (scalar_tensor_tensor FMAs, split across DVE and GPSIMD)
  S8 transpose back to channel-major, interleave R=2 phases -> o2c (128,84,164)
     + BN+SiLU of out_cv1
  S9 out_cv2 dw5x5 on the 160-grid (PE diag bf16 for most rows, DVE/GPSIMD STT
     for the rest) + BN+SiLU -> output channels 128..255
  S10 DMA o2c valid window -> output channels 0..127
"""

import numpy as np
import ml_dtypes

import concourse.bacc as bacc
import concourse.bass as bass
import concourse.tile as tile
from concourse import mybir
from concourse.bass_utils import run_bass_kernel_spmd
from concourse.masks import make_identity

F32 = mybir.dt.float32
F32R = mybir.dt.float32r
BF16 = mybir.dt.bfloat16
AF = mybir.ActivationFunctionType
ALU = mybir.AluOpType
AX = mybir.AxisListType

STOP_AFTER = None   # None | 'S4' | 'S6' | 'S8' (profiling aid)
EPS = 1e-5
WP = 84          # padded low-res width
ROWS = 52        # local input rows (valid image rows at local 6..45)
NKT = 42         # kt / o rows (local rows 5..46)
NZ = 44          # Z rows kept (local rows 4..47)
WO = 164         # padded hi-res width
HO = 84          # hi-res rows (output rows 80*half-2 .. 80*half+82)

# Per-rh CARAFE engine assignment: 'pe_gps' / 'pe_dve' = dg4 diag build on
# gpsimd / DVE + 9 wide PE matmuls; 'dve' = 4x STT FMA chains on DVE.
def _carafe_mode(rh):
    m = rh % 7
    if m in (0, 2, 4, 5):
        return 'pe_gps'
    if m in (1, 3):
        return 'pe_dve'
    return 'dve'


def _act(nc, out, in_, func, scale=1.0, bias=0.0):
    nc.scalar.activation(out=out, in_=in_, func=func, scale=scale, bias=bias)


def build_kernel():
    nc = bacc.Bacc("TRN2", target_bir_lowering=False, debug=False, num_devices=8)

    d = {}
    d["x_d"] = nc.declare_dram_parameter("x", [256, ROWS, WP], BF16, isOutput=False)
    d["edge_d"] = nc.declare_dram_parameter("edge", [128, 2], F32, isOutput=False)
    d["wdn1_d"] = nc.declare_dram_parameter("wdn1", [256, 32], BF16, isOutput=False)
    d["bdn1_d"] = nc.declare_dram_parameter("bdn1", [32, 2], F32, isOutput=False)
    d["ddn2_d"] = nc.declare_dram_parameter("ddn2", [25, 128, 128], BF16, isOutput=False)
    d["bdn2_d"] = nc.declare_dram_parameter("bdn2", [128, 2], F32, isOutput=False)
    d["wenc_d"] = nc.declare_dram_parameter("wenc", [9, 64, 18], BF16, isOutput=False)
    d["benc1_d"] = nc.declare_dram_parameter("benc1", [18, 2], F32, isOutput=False)
    d["denc2_d"] = nc.declare_dram_parameter("denc2", [25, 126, 126], BF16, isOutput=False)
    d["benc2_d"] = nc.declare_dram_parameter("benc2", [126, 2], F32, isOutput=False)
    d["wout1_d"] = nc.declare_dram_parameter("wout1", [256, 128], BF16, isOutput=False)
    d["bout1_d"] = nc.declare_dram_parameter("bout1", [128, 2], F32, isOutput=False)
    d["dout2_d"] = nc.declare_dram_parameter("dout2", [25, 128, 128], BF16, isOutput=False)
    d["wto2_d"] = nc.declare_dram_parameter("wto2", [128, 25], F32, isOutput=False)
    d["bout2_d"] = nc.declare_dram_parameter("bout2", [128, 2], F32, isOutput=False)
    d["out_d"] = nc.declare_dram_parameter("out", [256, 80, 160], F32, isOutput=True)

    with tile.TileContext(nc) as tc:
        _emit(nc, tc, d)
    nc.compile()
    return nc


def _emit(nc, tc, d):
    x_d, out_d = d["x_d"], d["out_d"]

    from contextlib import ExitStack
    ctx = ExitStack()
    with ctx:
        consts = ctx.enter_context(tc.tile_pool(name="consts", bufs=1))
        mid = ctx.enter_context(tc.tile_pool(name="mid", bufs=1))
        stage = ctx.enter_context(tc.tile_pool(name="stage", bufs=4))
        slab = ctx.enter_context(tc.tile_pool(name="slab", bufs=6))
        psA = ctx.enter_context(tc.tile_pool(name="psA", bufs=4, space="PSUM"))
        psT = ctx.enter_context(tc.tile_pool(name="psT", bufs=2, space="PSUM"))

        # ---- constants ---------------------------------------------------
        ident = consts.tile([128, 128], F32)
        make_identity(nc, ident[:])
        identb = consts.tile([128, 128], BF16)
        nc.gpsimd.tensor_copy(identb[:], ident[:])

        edge = consts.tile([128, 2], F32)
        nc.sync.dma_start(out=edge[:], in_=d["edge_d"][:])
        et, eb = edge[:, 0:1], edge[:, 1:2]

        wdn1 = consts.tile([128, 2, 32], BF16)
        nc.sync.dma_start(out=wdn1[:, 0, :], in_=d["wdn1_d"][0:128, :])
        nc.sync.dma_start(out=wdn1[:, 1, :], in_=d["wdn1_d"][128:256, :])
        bdn1 = consts.tile([32, 2], F32)
        nc.sync.dma_start(out=bdn1[:], in_=d["bdn1_d"][:])
        ddn2p = consts.tile([128, 25, 128], BF16)
        nc.sync.dma_start(out=ddn2p[:], in_=d["ddn2_d"][:].rearrange("t k m -> k t m"))
        bdn2 = consts.tile([128, 2], F32)
        nc.sync.dma_start(out=bdn2[:], in_=d["bdn2_d"][:])
        wenc = consts.tile([64, 9, 18], BF16)
        nc.sync.dma_start(out=wenc[:], in_=d["wenc_d"][:].rearrange("t k m -> k t m"))
        benc1 = consts.tile([18, 2], F32)
        nc.sync.dma_start(out=benc1[:], in_=d["benc1_d"][:])
        denc2p = consts.tile([126, 25, 126], BF16)
        nc.sync.dma_start(out=denc2p[:], in_=d["denc2_d"][:].rearrange("t k m -> k t m"))
        benc2 = consts.tile([126, 2], F32)
        nc.sync.dma_start(out=benc2[:], in_=d["benc2_d"][:])
        wout1 = consts.tile([128, 2, 128], BF16)
        nc.sync.dma_start(out=wout1[:, 0, :], in_=d["wout1_d"][0:128, :])
        nc.sync.dma_start(out=wout1[:, 1, :], in_=d["wout1_d"][128:256, :])
        bout1 = consts.tile([128, 2], F32)
        nc.sync.dma_start(out=bout1[:], in_=d["bout1_d"][:])
        dout2 = consts.tile([128, 25, 128], BF16)
        nc.sync.dma_start(out=dout2[:], in_=d["dout2_d"][:].rearrange("t k m -> k t m"))
        wto2 = consts.tile([128, 25], F32)
        nc.sync.dma_start(out=wto2[:], in_=d["wto2_d"][:])
        bout2 = consts.tile([128, 2], F32)
        nc.sync.dma_start(out=bout2[:], in_=d["bout2_d"][:])

        # mid-lived tensors
        kt = mid.tile([84, NKT, 36], F32)
        zt1 = mid.tile([84, NZ, 128], BF16)
        zt2 = mid.tile([84, NZ, 128], BF16)
        zt3 = mid.tile([84, NZ, 128], BF16)
        zts = {1: zt1, 2: zt2, 3: zt3}

        with tc.tile_pool(name="early", bufs=1) as early:
            x0 = early.tile([128, ROWS, WP], BF16)
            x1 = early.tile([128, ROWS, WP], BF16)
            nc.sync.dma_start(out=x0[:], in_=x_d[0:128])
            nc.sync.dma_start(out=x1[:], in_=x_d[128:256])
            down_t = early.tile([64, ROWS * WP + 8], BF16)
            down = down_t[:, 4:4 + ROWS * WP].rearrange("p (r w) -> p r w", w=WP)
            e = early.tile([36, ROWS, WP], F32)
            e1bf = early.tile([18, ROWS, WP], BF16)
            zc = early.tile([128, NZ, WP], BF16)
            nc.gpsimd.memset(down_t[:, 0:4], 0.0)
            nc.gpsimd.memset(down_t[:, 4 + ROWS * WP:], 0.0)

            def win(flat, p0, p1, off, rr, w):
                return flat[p0:p1, off:off + rr * w].rearrange(
                    "p (r w) -> p r w", w=w)

            y1 = down[0:32]
            nc.vector.memset(down[32:64, 0:2, :], 0.0)
            nc.vector.memset(down[32:64, 50:52, :], 0.0)


            # ---- S1: down_cv1 + BN + SiLU -------------------------------
            for c0 in range(0, ROWS, 6):
                rr = min(6, ROWS - c0)
                ps = psA.tile([128, 6, WP], F32, tag="ps")
                nc.tensor.matmul(ps[0:32, 0:rr, :], wdn1[:, 0, :],
                                 x0[:, c0:c0 + rr, :], start=True, stop=False)
                nc.tensor.matmul(ps[0:32, 0:rr, :], wdn1[:, 1, :],
                                 x1[:, c0:c0 + rr, :], start=False, stop=True)
                _act(nc, y1[:, c0:c0 + rr, :], ps[0:32, 0:rr, :], AF.Silu,
                     scale=bdn1[:, 0:1], bias=bdn1[:, 1:2])
            nc.vector.tensor_scalar_mul(y1[:, 0:6, :], y1[:, 0:6, :], et[0:32])
            nc.vector.tensor_scalar_mul(y1[:, 46:52, :], y1[:, 46:52, :], eb[0:32])
            nc.vector.memset(y1[:, :, 0:2], 0.0)
            nc.vector.memset(y1[:, :, 82:84], 0.0)

            # ---- S2: down_cv2 (diag bf16, 4 row-groups packed) ----------
            # group g (partitions 32g..32g+32) holds y1 rows [12g, 12g+16);
            # its outputs are rows [12g+2, 12g+14)
            y1s_t = early.tile([128, 16 * WP + 8], BF16)
            nc.gpsimd.memset(y1s_t[:, 0:4], 0.0)
            nc.gpsimd.memset(y1s_t[:, 4 + 16 * WP:], 0.0)
            for g in range(4):
                nc.sync.dma_start(
                    out=y1s_t[32 * g:32 * g + 32, 4:4 + 16 * WP],
                    in_=down_t[0:32, 4 + 12 * g * WP:4 + (12 * g + 16) * WP])
            for j in range(2):
                ps = psA.tile([128, 6, WP], F32, tag="ps")
                for t in range(25):
                    dh, dw = divmod(t, 5)
                    off = 4 + (6 * j + dh) * WP + (dw - 2)
                    nc.tensor.matmul(
                        ps[:, 0:6, :], ddn2p[:, t, :],
                        win(y1s_t, 0, 128, off, 6, WP),
                        start=(t == 0), stop=(t == 24))
                st = stage.tile([128, 6, WP], BF16, tag="sty2")
                _act(nc, st[:], ps[:], AF.Silu,
                     scale=bdn2[:, 0:1], bias=bdn2[:, 1:2])
                # edge masks: rows 2..5 (group0 chunk0 rows 0..3, et),
                # rows 46..49 (group3 chunk1 rows 2..5, eb)
                if j == 0:
                    nc.vector.tensor_scalar_mul(st[0:32, 0:4, :], st[0:32, 0:4, :],
                                                et[0:32])
                else:
                    nc.vector.tensor_scalar_mul(st[96:128, 2:6, :], st[96:128, 2:6, :],
                                                eb[96:128])
                nc.vector.memset(st[:, :, 0:2], 0.0)
                nc.vector.memset(st[:, :, 82:84], 0.0)
                for g in range(4):
                    nc.sync.dma_start(
                        out=down[32:64, 12 * g + 2 + 6 * j:12 * g + 8 + 6 * j, :],
                        in_=st[32 * g:32 * g + 32, :, :])

            # ---- S3: enc_cv1 (9 taps) + BN + SiLU -----------------------
            for c0 in range(3, 49, 6):
                rr = min(6, 49 - c0)
                ps = psA.tile([128, 6, WP], F32, tag="ps")
                for t in range(9):
                    dh, dw = divmod(t, 3)
                    off = 4 + (c0 - 1 + dh) * WP + (dw - 1)
                    nc.tensor.matmul(
                        ps[0:18, 0:rr, :], wenc[:, t, :],
                        win(down_t, 0, 64, off, rr, WP),
                        start=(t == 0), stop=(t == 8))
                _act(nc, e[0:18, c0:c0 + rr, :], ps[0:18, 0:rr, :], AF.Silu,
                     scale=benc1[:, 0:1], bias=benc1[:, 1:2])
            e1 = e[0:18]
            nc.vector.tensor_scalar_mul(e1[:, 3:6, :], e1[:, 3:6, :], et[0:18])
            nc.vector.tensor_scalar_mul(e1[:, 46:49, :], e1[:, 46:49, :], eb[0:18])
            nc.vector.memset(e1[:, :, 0:2], 0.0)
            nc.vector.memset(e1[:, :, 82:84], 0.0)
            nc.vector.memset(e1[:, 2:3, :], 0.0)
            nc.vector.memset(e1[:, 49:50, :], 0.0)
            nc.gpsimd.tensor_copy(e1bf[:, 2:50, :], e1[:, 2:50, :])

            # ---- S4: enc_cv2 (diag bf16, 7 row-groups packed) -----------
            # group g (partitions 18g..18g+18) holds e1 rows [6g+3, 6g+13);
            # outputs rows [6g+5, 6g+11)
            e1s_t = early.tile([128, 10 * WP + 8], BF16)
            nc.gpsimd.memset(e1s_t[:, 0:4], 0.0)
            nc.gpsimd.memset(e1s_t[:, 4 + 10 * WP:], 0.0)
            for g in range(7):
                nc.sync.dma_start(
                    out=e1s_t[18 * g:18 * g + 18, 4:4 + 10 * WP],
                    in_=e1bf[:, 6 * g + 3:6 * g + 13, :])
            ps = psA.tile([128, 6, WP], F32, tag="ps")
            for t in range(25):
                dh, dw = divmod(t, 5)
                off = 4 + dh * WP + (dw - 2)
                nc.tensor.matmul(
                    ps[0:126, 0:6, :], denc2p[:, t, :],
                    win(e1s_t, 0, 126, off, 6, WP),
                    start=(t == 0), stop=(t == 24))
            st = stage.tile([126, 6, WP], F32, tag="ste2")
            _act(nc, st[:], ps[0:126, :, :], AF.Silu,
                 scale=benc2[:, 0:1], bias=benc2[:, 1:2])
            for g in range(7):
                nc.sync.dma_start(out=e[18:36, 6 * g + 5:6 * g + 11, :],
                                  in_=st[18 * g:18 * g + 18, :, :])

            if STOP_AFTER == 'S4':
                return
            # ---- S5: softmax -> kt (pixel-major, per row) ---------------
            for rh in range(NKT):
                row = 5 + rh
                pt = psT.tile([80, 36], F32, tag="pt")
                nc.tensor.transpose(pt[:], e[0:36, row, 2:82], ident[0:36, 0:36])
                erow = stage.tile([80, 36], F32, tag="erow")
                _act(nc, erow[:], pt[0:80, 0:36], AF.Exp)
                srow = stage.tile([80, 4], F32, tag="srow")
                nc.vector.tensor_reduce(
                    srow[:], erow[:].rearrange("w (k r) -> w r k", k=9),
                    axis=AX.X, op=ALU.add)
                nc.vector.reciprocal(srow[:], srow[:])
                nc.vector.tensor_tensor(
                    kt[0:80, rh, :].rearrange("w (k r) -> w k r", k=9),
                    erow[:].rearrange("w (k r) -> w k r", k=9),
                    srow[:].unsqueeze(1).to_broadcast((80, 9, 4)),
                    op=ALU.mult)

            # ---- S6: Z + shifted transposed copies ----------------------
            for c0 in range(0, NZ, 6):
                rr = min(6, NZ - c0)
                ps = psA.tile([128, 6, WP], F32, tag="ps")
                nc.tensor.matmul(ps[:, 0:rr, :], wout1[:, 0, :],
                                 x0[:, 4 + c0:4 + c0 + rr, :], start=True, stop=False)
                nc.tensor.matmul(ps[:, 0:rr, :], wout1[:, 1, :],
                                 x1[:, 4 + c0:4 + c0 + rr, :], start=False, stop=True)
                _act(nc, zc[:, c0:c0 + rr, :], ps[:, 0:rr, :], AF.Copy)
            for zr in range(NZ):
                for di, dd in enumerate((1, 2, 3)):
                    pt = psT.tile([80, 128], BF16, tag="pt")
                    nc.tensor.transpose(pt[:], zc[:, zr, dd:dd + 80], identb[:])
                    if (zr * 3 + di) % 2 == 0:
                        _act(nc, zts[dd][0:80, zr, :], pt[0:80, 0:128], AF.Copy)
                    else:
                        nc.vector.tensor_copy(zts[dd][0:80, zr, :], pt[0:80, 0:128])

        if STOP_AFTER == 'S6':
            return
        # early pool freed here
        from contextlib import ExitStack as _ES
        lctx = _ES()
        with lctx:
            late = lctx.enter_context(tc.tile_pool(name="late", bufs=1))
            dgp = lctx.enter_context(tc.tile_pool(name="dgp", bufs=4))
            rowp = lctx.enter_context(tc.tile_pool(name="rowp", bufs=4))
            o2bf_t = late.tile([128, HO * WO + 8], BF16)
            o2bf = o2bf_t[:, 4:4 + HO * WO].rearrange("p (r w) -> p r w", w=WO)
            nc.gpsimd.memset(o2bf_t[:, 0:4], 0.0)
            nc.gpsimd.memset(o2bf_t[:, 4 + HO * WO:], 0.0)
            nc.vector.memset(o2bf[:, :, 0:2], 0.0)
            nc.vector.memset(o2bf[:, :, 162:164], 0.0)

            def win9(off, rr):
                return o2bf_t[:, off:off + rr * WO].rearrange(
                    "p (r w) -> p r w", w=WO)

            # ---- S9 chunk: out_cv2 dw5x5 rows [g0-2, g0-2+rr) of out ----
            def emit_s9(c):
                g0 = 2 + 3 * c
                rr = min(3, 82 - g0)
                ps = psA.tile([128, 3, WO], F32, tag="ps")
                for t in range(25):
                    dh, dw = divmod(t, 5)
                    off = 4 + (g0 - 2 + dh) * WO + (dw - 2)
                    nc.tensor.matmul(
                        ps[:, 0:rr, :], dout2[:, t, :], win9(off, rr),
                        start=(t == 0), stop=(t == 24))
                st = stage.tile([128, 3, WO], F32, tag="st9")
                _act(nc, st[:, 0:rr, :], ps[:, 0:rr, :], AF.Silu,
                     scale=bout2[:, 0:1], bias=bout2[:, 1:2])
                nc.sync.dma_start(out=out_d[128:256, g0 - 2:g0 - 2 + rr, :],
                                  in_=st[:, 0:rr, 2:162])

            # ---- S7/S8: CARAFE (per-rh engine mode) + interleaved S9 ----
            next_c = 0
            for rh in range(NKT):
                while next_c < 27 and rh >= (3 * next_c + 8) // 2:
                    emit_s9(next_c)
                    next_c += 1
                mode = _carafe_mode(rh)
                if mode == 'dve':
                    # 4x (TS + 8 STT FMA chain) pixel-major + PE transposes
                    ptd = psT.tile([128, 4, 80], BF16, tag="ptx")
                    for r in range(4):
                        acc = slab.tile([80, 128], BF16, tag="acc")
                        for k in range(9):
                            dh, dwp = divmod(k, 3)
                            src = zts[1 + dwp][0:80, rh + dh, :]
                            sc = kt[0:80, rh, 4 * k + r:4 * k + r + 1]
                            if k == 0:
                                nc.vector.tensor_scalar_mul(acc[:], src, sc)
                            else:
                                nc.vector.scalar_tensor_tensor(
                                    out=acc[:], in0=src, scalar=sc, in1=acc[:],
                                    op0=ALU.mult, op1=ALU.add)
                        nc.tensor.transpose(ptd[:, r, :], acc[:],
                                            identb[0:80, 0:80])
                    pr = ptd
                else:
                    # diag build (dg4[w, r, k, w'] = kt[w, 4k+r]*I[w,w'])
                    # on gpsimd or DVE + 9 wide PE matmuls
                    dg4 = dgp.tile([80, 4, 9, 80], BF16, tag="dg4")
                    eng = nc.gpsimd if mode == 'pe_gps' else nc.vector
                    eng.tensor_tensor(
                        dg4[:],
                        identb[0:80, 0:80].unsqueeze(1).unsqueeze(1)
                            .to_broadcast((80, 4, 9, 80)),
                        kt[0:80, rh, :].rearrange("w (k r) -> w r k", k=9)
                            .unsqueeze(3).to_broadcast((80, 4, 9, 80)),
                        op=ALU.mult)
                    ptp = psT.tile([128, 4, 80], F32, tag="ptx")
                    for k in range(9):
                        dh, dwp = divmod(k, 3)
                        nc.tensor.matmul(
                            ptp[:], zts[1 + dwp][0:80, rh + dh, :],
                            dg4[:, :, k, :], start=(k == 0), stop=(k == 8))
                    pr = ptp
                # drain: BN+SiLU of out_cv1, interleave (w, r2) -> hi cols
                o2p = rowp.tile([128, 2, 160], F32, tag="o2p")
                _act(nc, o2p[:].rearrange("p h (w q) -> p h q w", q=2),
                     pr[0:128].rearrange("p (h q) w -> p h q w", q=2), AF.Silu,
                     scale=bout1[:, 0:1], bias=bout1[:, 1:2])
                if rh == 0:
                    nc.vector.tensor_scalar_mul(o2p[:], o2p[:], et)
                if rh == NKT - 1:
                    nc.vector.tensor_scalar_mul(o2p[:], o2p[:], eb)
                # bf16 copy for S9's window reads
                nc.scalar.activation(out=o2bf[:, 2 * rh:2 * rh + 2, 2:162],
                                     in_=o2p[:], func=AF.Copy)
                # output channels 0..127 (valid rows only)
                if 1 <= rh <= 40:
                    nc.sync.dma_start(out=out_d[0:128, 2 * rh - 2:2 * rh, :],
                                      in_=o2p[:])
            while next_c < 27:
                emit_s9(next_c)
                next_c += 1


# ---------------------------------------------------------------------------
# host side
# ---------------------------------------------------------------------------

_NC_CACHE = {}


def _get_nc():
    if "nc" not in _NC_CACHE:
        _NC_CACHE["nc"] = build_kernel()
    return _NC_CACHE["nc"]


def _bn2(g, b, m, v):
    inv = (g / np.sqrt(v + EPS)).astype(np.float32)
    beta = (b - m * inv).astype(np.float32)
    return np.stack([inv, beta], axis=1).astype(np.float32)


def _diag_taps(w, c, rep=1):
    taps = np.tile(w.reshape(c, 25).T, (1, rep))      # (25, c*rep)
    n = c * rep
    out = np.zeros((25, n, n), np.float32)
    idx = np.arange(n)
    out[:, idx, idx] = taps
    return out.astype(ml_dtypes.bfloat16)


def _tile_bn(bn, rep):
    return np.tile(bn, (rep, 1))


def prep_in_maps(inputs):
    inp = {k: np.asarray(v) for k, v in inputs.items()}
    x = inp["x"].astype(np.float32)

    common = dict(
        wdn1=np.ascontiguousarray(inp["down_cv1_w"].reshape(32, 256).T).astype(ml_dtypes.bfloat16),
        bdn1=_bn2(inp["down_cv1_g"], inp["down_cv1_b"], inp["down_cv1_m"], inp["down_cv1_v"]),
        ddn2=_diag_taps(inp["down_cv2_w"], 32, rep=4),
        bdn2=_tile_bn(_bn2(inp["down_cv2_g"], inp["down_cv2_b"], inp["down_cv2_m"], inp["down_cv2_v"]), 4),
        wenc=np.ascontiguousarray(inp["enc_cv1_w"].reshape(18, 64, 9).transpose(2, 1, 0)).astype(ml_dtypes.bfloat16),
        benc1=_bn2(inp["enc_cv1_g"], inp["enc_cv1_b"], inp["enc_cv1_m"], inp["enc_cv1_v"]),
        denc2=_diag_taps(inp["enc_cv2_w"], 18, rep=7),
        benc2=_tile_bn(_bn2(inp["enc_cv2_g"], inp["enc_cv2_b"], inp["enc_cv2_m"], inp["enc_cv2_v"]), 7),
        wout1=np.ascontiguousarray(inp["out_cv1_w"].reshape(128, 256).T).astype(ml_dtypes.bfloat16),
        bout1=_bn2(inp["out_cv1_g"], inp["out_cv1_b"], inp["out_cv1_m"], inp["out_cv1_v"]),
        dout2=_diag_taps(inp["out_cv2_w"], 128),
        wto2=np.ascontiguousarray(inp["out_cv2_w"].reshape(128, 25)).astype(np.float32),
        bout2=_bn2(inp["out_cv2_g"], inp["out_cv2_b"], inp["out_cv2_m"], inp["out_cv2_v"]),
    )

    in_maps = []
    for s in range(8):
        n, half = s // 2, s % 2
        h0 = 40 * half
        xs = np.zeros((256, ROWS, WP), ml_dtypes.bfloat16)
        src_lo = max(0, h0 - 6)
        src_hi = min(80, h0 + 46)
        xs[:, src_lo - (h0 - 6):src_hi - (h0 - 6), 2:82] = x[n, :, src_lo:src_hi, :]
        edge = np.zeros((128, 2), np.float32)
        edge[:, 0] = 0.0 if half == 0 else 1.0
        edge[:, 1] = 1.0 if half == 0 else 0.0
        in_maps.append(dict(x=xs, edge=edge, **common))
    return in_maps


def kernel(**inputs):
    in_maps = prep_in_maps(inputs)
    nc = _get_nc()
    res = run_bass_kernel_spmd(nc, in_maps, list(range(8)))
    _NC_CACHE["last_result"] = res

    out = np.empty((4, 256, 160, 160), np.float32)
    for s in range(8):
        n, half = s // 2, s % 2
        out[n, :, 80 * half:80 * half + 80, :] = res.results[s]["out"]
    return out

